# revision 26
# baseline (speedup 1.0000x reference)
"""AFT-Full (Attention Free Transformer) on 8 Trainium2 NeuronCores.

Math (per batch b):
  Q = x@Wq+bq, K = x@Wk+bk, V = x@Wv+bv          (per-head reshape is a no-op
  num = ew @ (exp(K) * V), den = ew @ exp(K)      because ew is shared by all
  out = (sigmoid(Q) * num / den) @ Wo + bo        heads: ew = exp(wbias))

Identities used:
  - with biases bk, bv: num/den = num0/den0 + bv and bk cancels entirely.
  - ew = 1 + delta with |delta| <= 0.04 (wbias is xavier-small), so
    num = colsum(eKV) + delta @ eKV, den = colsum(eK) + delta @ eK.
    This lets delta (scaled x256) and eK/eKV live in fp8e4m3 while ew == 1
    to machine precision would have destroyed fp8's mantissa.

Sharding: 8 cores = 4 batches x 2 head-groups (512 features each).  Each
core computes a partial [T, D] output; the host adds the two group partials
per batch plus bo.

Precision plan (validated vs reference in numpy, rel err ~1.2e-2 global):
  - K/V projections + AFT delta-matmuls: fp8e4m3 with DoubleRow (2x rate).
    Scales: W x16 (away from denorms), eK /2, eKV /8, delta x256.
  - Q projection + out projection: bf16 (full rate).
  - All PSUM accumulation f32; epilogues f32; output f32.

DoubleRow pairs the contraction dim: lhsT/rhs are [128, 2, free] APs and
out = sum_e lhsT[:,e,:].T @ rhs[:,e,:].  The host pre-interleaves x, W and
delta into that paired layout; eK/eKV pair tiles are filled by the
projection drains (t-tile tt -> pair tt//2, slot tt%2).

Column sums are one ones-vector matmul pass -> [1, 512] PSUM, scattered to
per-partition [128, 4] vectors by small DMAs, then injected as ACT biases in
the AFT epilogue: ratio = (4/256 * dnum + 4*cn) / (1/256 * dden + cd) which
equals num/den exactly for the chosen scales (EKVS/EKS = 4).
"""

import numpy as np
import ml_dtypes

B, T, D, H = 4, 2048, 1024, 16
G = 2                  # head-groups (cores = B * G)
JG = D // G            # 512 features per group
NCORES = 8
P = 128                # partition tile
NDT = D // P           # 8  d-tiles
NDP = NDT // 2         # 4  paired d-tiles (DoubleRow)
NTT = T // P           # 16 t-tiles / s-tiles
NSP = NTT // 2         # 8  paired s-tiles
NJT = JG // P          # 4  j-tiles per group
NC_CHUNK = 512         # matmul moving free-dim (one PSUM bank of f32)
NTC = T // NC_CHUNK    # 4  t-chunks
NIC = D // NC_CHUNK    # 2  i-chunks of the final output

WSCALE = 16.0          # W pre-scale (host)
EKS = 2.0              # eK stored as eK/EKS
EKVS = 8.0             # eKV stored as eKV/EKVS
DSCALE = 256.0         # delta stored as delta*DSCALE
LN_EKS = float(np.log(EKS))

_NC = None             # cached compiled Bass graph


def _build():
    from concourse import bacc, mybir, tile

    dt = mybir.dt
    bf = dt.bfloat16
    f8 = dt.float8e4
    f32 = dt.float32
    Alu = mybir.AluOpType
    Act = mybir.ActivationFunctionType
    DR = mybir.MatmulPerfMode.DoubleRow

    nc = bacc.Bacc(target_bir_lowering=False)

    xTb_d = nc.declare_dram_parameter("xTb", [D, T], bf, isOutput=False)
    x8_d = nc.declare_dram_parameter("x8p", [NDP * P, 2 * T], f8, isOutput=False)
    wk_d = nc.declare_dram_parameter("wk8p", [NDP * P, 2 * JG], f8, isOutput=False)
    wv_d = nc.declare_dram_parameter("wv8p", [NDP * P, 2 * JG], f8, isOutput=False)
    wq_d = nc.declare_dram_parameter("wq", [D, JG], bf, isOutput=False)
    wo_d = nc.declare_dram_parameter("wo", [JG, D], bf, isOutput=False)
    d8_d = nc.declare_dram_parameter("d8p", [NSP * P, 2 * T], f8, isOutput=False)
    bq_d = nc.declare_dram_parameter("bqT", [P, NJT], f32, isOutput=False)
    bv_d = nc.declare_dram_parameter("bvT", [P, NJT], f32, isOutput=False)
    out_d = nc.declare_dram_parameter("out", [T, D], f32, isOutput=True)

    with tile.TileContext(nc) as tc:
        with (
            tc.tile_pool(name="const", bufs=1) as cp,
            tc.tile_pool(name="ew", bufs=24) as ewp,
            tc.tile_pool(name="ps", bufs=8, space="PSUM") as pp,
            tc.tile_pool(name="tmp", bufs=4) as tp,
        ):
            # ---- constant loads (first K-proj deps hoisted to the top) ----
            x8_0 = cp.tile([P, 2, T], f8, tag="x8_0", name="x8_0")
            nc.sync.dma_start(x8_0[:], x8_d[0:P, :].rearrange("p (e t) -> p e t", e=2))
            wk8_0 = cp.tile([P, 2, JG], f8, tag="wk8_0", name="wk8_0")
            nc.sync.dma_start(wk8_0[:], wk_d[0:P, :].rearrange("p (e t) -> p e t", e=2))
            bq_sb = cp.tile([P, NJT], f32, tag="bq", name="bq")
            bv_sb = cp.tile([P, NJT], f32, tag="bv", name="bv")
            nc.sync.dma_start(bq_sb[:], bq_d[:])
            nc.sync.dma_start(bv_sb[:], bv_d[:])
            negln = cp.tile([P, 1], f32, tag="negln", name="negln")
            nc.vector.memset(negln[:], -LN_EKS)

            # paired fp8 x / W tiles (interleave loads: x pair, wk pair, ...)
            x8, wk8, wv8 = [x8_0], [wk8_0], []
            for i in range(1, NDP):
                tx = cp.tile([P, 2, T], f8, tag=f"x8_{i}", name=f"x8_{i}")
                nc.sync.dma_start(tx[:], x8_d[i * P:(i + 1) * P, :].rearrange(
                    "p (e t) -> p e t", e=2))
                x8.append(tx)
                tk = cp.tile([P, 2, JG], f8, tag=f"wk8_{i}", name=f"wk8_{i}")
                nc.sync.dma_start(tk[:], wk_d[i * P:(i + 1) * P, :].rearrange(
                    "p (e t) -> p e t", e=2))
                wk8.append(tk)
            for i in range(NDP):
                tv = cp.tile([P, 2, JG], f8, tag=f"wv8_{i}", name=f"wv8_{i}")
                nc.sync.dma_start(tv[:], wv_d[i * P:(i + 1) * P, :].rearrange(
                    "p (e t) -> p e t", e=2))
                wv8.append(tv)

            xTb = []
            for d in range(NDT):
                t_ = cp.tile([P, T], bf, tag=f"xTb{d}", name=f"xTb{d}")
                nc.sync.dma_start(t_[:], xTb_d[d * P:(d + 1) * P, :])
                xTb.append(t_)
            wq = []
            for d in range(NDT):
                t_ = cp.tile([P, JG], bf, tag=f"wq{d}", name=f"wq{d}")
                nc.sync.dma_start(t_[:], wq_d[d * P:(d + 1) * P, :])
                wq.append(t_)
            wo = []
            for i in range(NJT):
                t_ = cp.tile([P, D], bf, tag=f"wo{i}", name=f"wo{i}")
                nc.sync.dma_start(t_[:], wo_d[i * P:(i + 1) * P, :])
                wo.append(t_)

            eK8 = [cp.tile([P, 2, JG], f8, tag=f"eK8_{i}", name=f"eK8_{i}")
                   for i in range(NSP)]
            eKV8 = [cp.tile([P, 2, JG], f8, tag=f"eKV8_{i}", name=f"eKV8_{i}")
                    for i in range(NSP)]
            QT = [cp.tile([P, T], f32, tag=f"QT{j}", name=f"QT{j}") for j in range(NJT)]
            aftT = [cp.tile([P, T], bf, tag=f"aftT{j}", name=f"aftT{j}")
                    for j in range(NJT)]
            cn4_sb = cp.tile([P, NJT], f32, tag="cn4", name="cn4")
            cd_sb = cp.tile([P, NJT], f32, tag="cd", name="cd")
            cn_acc = cp.tile([P, JG], f32, tag="cn_acc", name="cn_acc")
            cd_acc = cp.tile([P, JG], f32, tag="cd_acc", name="cd_acc")
            cn_red = cp.tile([P, JG], f32, tag="cn_red", name="cn_red")
            cd_red = cp.tile([P, JG], f32, tag="cd_red", name="cd_red")

            # ---- K/V projections: fp8 DoubleRow, [s, j] natural layout ----
            # PSUM holds 16*K_true (W was pre-scaled x16).  K leads V by 4
            # t-tiles so V's first matmuls don't stall on the wv8 DMAs.
            def emit_kproj(tt):
                i8, e = divmod(tt, 2)
                ps_k = pp.tile([P, JG], f32, tag="ps", name="psv")
                for i in range(NDP):
                    nc.tensor.matmul(
                        ps_k[:],
                        x8[i][:, :, tt * P:(tt + 1) * P],
                        wk8[i][:],
                        start=(i == 0), stop=(i == NDP - 1),
                        perf_mode=DR,
                    )
                # eK8 = exp(psum/16)/EKS = exp(psum/16 - ln EKS)
                nc.scalar.activation(eK8[i8][:, e, :], ps_k[:], Act.Exp,
                                     bias=negln[:], scale=1.0 / WSCALE)
                if tt == 0:
                    nc.vector.tensor_copy(cd_acc[:], eK8[i8][:, e, :])
                else:
                    nc.vector.tensor_tensor(cd_acc[:], cd_acc[:],
                                            eK8[i8][:, e, :], Alu.add)

            def emit_vproj(tt):
                i8, e = divmod(tt, 2)
                ps_v = pp.tile([P, JG], f32, tag="ps", name="psv")
                for i in range(NDP):
                    nc.tensor.matmul(
                        ps_v[:],
                        x8[i][:, :, tt * P:(tt + 1) * P],
                        wv8[i][:],
                        start=(i == 0), stop=(i == NDP - 1),
                        perf_mode=DR,
                    )
                # eKV8 = eK8 * EKS * (psum/16) / EKVS = (psum/64) * eK8
                nc.vector.scalar_tensor_tensor(
                    eKV8[i8][:, e, :], ps_v[:],
                    EKS / (WSCALE * EKVS), eK8[i8][:, e, :],
                    Alu.mult, Alu.mult,
                )
                if tt == 0:
                    nc.vector.tensor_copy(cn_acc[:], eKV8[i8][:, e, :])
                else:
                    nc.vector.tensor_tensor(cn_acc[:], cn_acc[:],
                                            eKV8[i8][:, e, :], Alu.add)

            KV_LEAD = 4
            for tt in range(KV_LEAD):
                emit_kproj(tt)
            for tt in range(NTT):
                if tt + KV_LEAD < NTT:
                    emit_kproj(tt + KV_LEAD)
                emit_vproj(tt)

            # ---- column sums on gpsimd (PE stays on matmuls) --------------
            # per-slice partials accumulated on DVE above; one cross-partition
            # reduce each here, then scatter row 0 into per-partition vectors.
            from concourse import bass_isa
            nc.gpsimd.partition_all_reduce(cn_red[:], cn_acc[:], P,
                                           bass_isa.ReduceOp.add)
            nc.gpsimd.partition_all_reduce(cd_red[:], cd_acc[:], P,
                                           bass_isa.ReduceOp.add)
            cn_row = tp.tile([1, JG], f32, tag="cnr", name="cnr", bufs=1)
            # nn uses scale 4/DSCALE with bias 4*cn; dd uses 1/DSCALE with cd
            nc.scalar.mul(cn_row[:], cn_red[0:1, :], EKVS / EKS)
            for m in range(NJT):
                nc.sync.dma_start(cn4_sb[:, m:m + 1],
                                  cn_row[0:1, m * P:(m + 1) * P])
                nc.sync.dma_start(cd_sb[:, m:m + 1],
                                  cd_red[0:1, m * P:(m + 1) * P])

            # ---- Q projection (bf16), transposed: QT[j, t] = sigmoid(.) ---
            for jt in range(NJT):
                for c in range(NTC):
                    ps_q = pp.tile([P, NC_CHUNK], f32, tag="ps", name="psc")
                    for d in range(NDT):
                        nc.tensor.matmul(
                            ps_q[:],
                            wq[d][:, jt * P:(jt + 1) * P],
                            xTb[d][:, c * NC_CHUNK:(c + 1) * NC_CHUNK],
                            start=(d == 0), stop=(d == NDT - 1),
                        )
                    nc.scalar.activation(
                        QT[jt][:, c * NC_CHUNK:(c + 1) * NC_CHUNK],
                        ps_q[:], Act.Sigmoid, bias=bq_sb[:, jt:jt + 1],
                    )

            # ---- AFT delta matmuls (fp8 DoubleRow) + epilogue + outproj ---
            def emit_outproj(c):
                for tt in range(4 * c, 4 * (c + 1)):
                    for ic in range(NIC):
                        ps_y = pp.tile([P, NC_CHUNK], f32, tag="ps", name="psc")
                        for jt in range(NJT):
                            nc.tensor.matmul(
                                ps_y[:],
                                aftT[jt][:, tt * P:(tt + 1) * P],
                                wo[jt][:, ic * NC_CHUNK:(ic + 1) * NC_CHUNK],
                                start=(jt == 0), stop=(jt == NJT - 1),
                            )
                        y_sb = tp.tile([P, NC_CHUNK], f32, tag="y", name="ysb", bufs=6)
                        if (tt * NIC + ic) % 2 == 0:
                            nc.vector.tensor_copy(y_sb[:], ps_y[:])
                        else:
                            nc.scalar.copy(y_sb[:], ps_y[:])
                        nc.sync.dma_start(
                            out_d[tt * P:(tt + 1) * P,
                                  ic * NC_CHUNK:(ic + 1) * NC_CHUNK],
                            y_sb[:],
                        )

            for c in range(NTC):
                dw_c = []
                for i in range(NSP):
                    dw_t = ewp.tile([P, 2, NC_CHUNK], f8, tag="ewt", name="ewt")
                    nc.sync.dma_start(
                        dw_t[:],
                        d8_d[i * P:(i + 1) * P, :].rearrange(
                            "p (e t) -> p e t", e=2)[:, :, c * NC_CHUNK:(c + 1) * NC_CHUNK],
                    )
                    dw_c.append(dw_t)
                for half in range(2):
                    ms = (2 * half, 2 * half + 1)
                    ps_n = {m: pp.tile([P, NC_CHUNK], f32, tag="ps", name="psc")
                            for m in ms}
                    ps_dn = {m: pp.tile([P, NC_CHUNK], f32, tag="ps", name="psc")
                             for m in ms}
                    for i in range(NSP):
                        for m in ms:
                            nc.tensor.matmul(
                                ps_n[m][:],
                                eKV8[i][:, :, m * P:(m + 1) * P],
                                dw_c[i][:],
                                start=(i == 0), stop=(i == NSP - 1),
                                perf_mode=DR,
                            )
                            nc.tensor.matmul(
                                ps_dn[m][:],
                                eK8[i][:, :, m * P:(m + 1) * P],
                                dw_c[i][:],
                                start=(i == 0), stop=(i == NSP - 1),
                                perf_mode=DR,
                            )
                    for m in ms:
                        # nn = 4*num/EKVS = dnum*(4/DSCALE) + 4*cn
                        # dd = den/EKS    = dden*(1/DSCALE) + cd
                        # nn/dd = num/den exactly (EKVS/EKS = 4)
                        nn = tp.tile([P, NC_CHUNK], f32, tag="nn", name="nn", bufs=3)
                        nc.scalar.activation(nn[:], ps_n[m][:], Act.Identity,
                                             bias=cn4_sb[:, m:m + 1],
                                             scale=(EKVS / EKS) / DSCALE)
                        dd = tp.tile([P, NC_CHUNK], f32, tag="dd", name="dd", bufs=3)
                        nc.scalar.activation(dd[:], ps_dn[m][:], Act.Identity,
                                             bias=cd_sb[:, m:m + 1],
                                             scale=1.0 / DSCALE)
                        rcp = tp.tile([P, NC_CHUNK], f32, tag="rcp", name="rcp", bufs=3)
                        nc.vector.reciprocal_approx_fast(rcp[:], dd[:])
                        prod = tp.tile([P, NC_CHUNK], f32, tag="prod", name="prod", bufs=3)
                        nc.vector.tensor_tensor(prod[:], nn[:], rcp[:], Alu.mult)
                        # aftT = (num/den + bv) * sigmoid(QT)
                        nc.vector.scalar_tensor_tensor(
                            aftT[m][:, c * NC_CHUNK:(c + 1) * NC_CHUNK],
                            prod[:], bv_sb[:, m:m + 1],
                            QT[m][:, c * NC_CHUNK:(c + 1) * NC_CHUNK],
                            Alu.add, Alu.mult,
                        )
                if c > 0:
                    emit_outproj(c - 1)
            emit_outproj(NTC - 1)

    nc.compile()
    return nc


def _get_nc():
    global _NC
    if _NC is None:
        _NC = _build()
    return _NC


def _pair(a, npair):
    """[npair*256, W] -> [npair*128, 2*W] DoubleRow-paired layout."""
    w = a.shape[1]
    return np.ascontiguousarray(
        a.reshape(npair, 2, P, w).transpose(0, 2, 1, 3).reshape(npair * P, 2 * w))


def make_in_maps(inputs):
    bf16 = ml_dtypes.bfloat16
    f8 = ml_dtypes.float8_e4m3
    x = np.asarray(inputs["x"], dtype=np.float32)
    Wq = np.asarray(inputs["Wq"], dtype=np.float32)
    Wk = np.asarray(inputs["Wk"], dtype=np.float32)
    Wv = np.asarray(inputs["Wv"], dtype=np.float32)
    Wo = np.asarray(inputs["Wo"], dtype=np.float32)
    bq = np.asarray(inputs["bq"], dtype=np.float32)
    bv = np.asarray(inputs["bv"], dtype=np.float32)
    wbias = np.asarray(inputs["wbias"], dtype=np.float32)
    # bk cancels in num/den; bq, bv, bo handled explicitly.

    d8p = _pair(((np.exp(wbias) - 1.0) * DSCALE).T.astype(f8).astype(np.float32),
                NSP).astype(f8)

    in_maps = []
    for c in range(NCORES):
        b, g = divmod(c, G)
        sl = slice(g * JG, (g + 1) * JG)
        xT = np.ascontiguousarray(x[b].T)
        in_maps.append({
            "xTb": xT.astype(bf16),
            "x8p": _pair(xT.astype(f8).astype(np.float32), NDP).astype(f8),
            "wk8p": _pair(np.ascontiguousarray(Wk[:, sl]) * WSCALE, NDP).astype(f8),
            "wv8p": _pair(np.ascontiguousarray(Wv[:, sl]) * WSCALE, NDP).astype(f8),
            "wq": np.ascontiguousarray(Wq[:, sl]).astype(bf16),
            "wo": np.ascontiguousarray(Wo[sl, :]).astype(bf16),
            "d8p": d8p,
            "bqT": np.ascontiguousarray(bq[sl].reshape(NJT, P).T),
            "bvT": np.ascontiguousarray(bv[sl].reshape(NJT, P).T),
        })
    return in_maps


def kernel(**inputs):
    import os
    import sys
    # The bass kernel needs the axon/neuron jax backend (8 NeuronCores). If a
    # harness pinned jax to cpu for the reference and jax isn't imported yet,
    # unpin it for this process.
    if "jax" not in sys.modules and os.environ.get("JAX_PLATFORMS") == "cpu":
        del os.environ["JAX_PLATFORMS"]

    from concourse.bass_utils import run_bass_kernel_spmd

    bo = np.asarray(inputs["bo"], dtype=np.float32)
    in_maps = make_in_maps(inputs)
    res = run_bass_kernel_spmd(_get_nc(), in_maps, core_ids=list(range(NCORES)))
    parts = [res.results[c]["out"] for c in range(NCORES)]
    out = np.empty((B, T, D), dtype=np.float32)
    for b in range(B):
        out[b] = parts[G * b] + parts[G * b + 1] + bo[None, :]
    return out


# revision 29
# speedup vs baseline: 1.1626x; 1.1626x over previous
"""AFT-Full (Attention Free Transformer) on 8 Trainium2 NeuronCores.

Math (per batch b):
  Q = x@Wq+bq, K = x@Wk+bk, V = x@Wv+bv          (per-head reshape is a no-op
  num = ew @ (exp(K) * V), den = ew @ exp(K)      because ew is shared by all
  out = (sigmoid(Q) * num / den) @ Wo + bo        heads: ew = exp(wbias))

Identities used:
  - with biases bk, bv: num/den = num0/den0 + bv and bk cancels entirely.
  - ew = 1 + delta with |delta| <= 0.04 (wbias is xavier-small), so
    num = colsum(eKV) + delta @ eKV, den = colsum(eK) + delta @ eK.
    This lets delta (scaled x256) and eK/eKV live in fp8e4m3 while ew == 1
    to machine precision would have destroyed fp8's mantissa.

Sharding: 8 cores = 4 batches x 2 head-groups (512 features each).  Each
core computes a partial [T, D] output; the host adds the two group partials
per batch plus bo.

Precision plan (validated vs reference in numpy, rel err ~1.2e-2 global):
  - K/V projections + AFT delta-matmuls: fp8e4m3 with DoubleRow (2x rate).
    Scales: W x16 (away from denorms), eK /2, eKV /8, delta x256.
  - Q projection + out projection: bf16 (full rate).
  - All PSUM accumulation f32; epilogues f32; output f32.

DoubleRow pairs the contraction dim: lhsT/rhs are [128, 2, free] APs and
out = sum_e lhsT[:,e,:].T @ rhs[:,e,:].  The host pre-interleaves x, W and
delta into that paired layout; eK/eKV pair tiles are filled by the
projection drains (t-tile tt -> pair tt//2, slot tt%2).

Column sums are one ones-vector matmul pass -> [1, 512] PSUM, scattered to
per-partition [128, 4] vectors by small DMAs, then injected as ACT biases in
the AFT epilogue: ratio = (4/256 * dnum + 4*cn) / (1/256 * dden + cd) which
equals num/den exactly for the chosen scales (EKVS/EKS = 4).
"""

import numpy as np
import ml_dtypes

B, T, D, H = 4, 2048, 1024, 16
G = 2                  # head-groups (cores = B * G)
JG = D // G            # 512 features per group
NCORES = 8
P = 128                # partition tile
NDT = D // P           # 8  d-tiles
NDP = NDT // 2         # 4  paired d-tiles (DoubleRow)
NTT = T // P           # 16 t-tiles / s-tiles
NSP = NTT // 2         # 8  paired s-tiles
NJT = JG // P          # 4  j-tiles per group
NC_CHUNK = 512         # matmul moving free-dim (one PSUM bank of f32)
NTC = T // NC_CHUNK    # 4  t-chunks
NIC = D // NC_CHUNK    # 2  i-chunks of the final output

WSCALE = 16.0          # W pre-scale (host)
EKS = 2.0              # eK stored as eK/EKS
EKVS = 8.0             # eKV stored as eKV/EKVS
DSCALE = 256.0         # delta stored as delta*DSCALE
LN_EKS = float(np.log(EKS))

_NC = None             # cached compiled Bass graph


def _build():
    from concourse import bacc, mybir, tile

    dt = mybir.dt
    bf = dt.bfloat16
    f8 = dt.float8e4
    f32 = dt.float32
    Alu = mybir.AluOpType
    Act = mybir.ActivationFunctionType
    DR = mybir.MatmulPerfMode.DoubleRow

    nc = bacc.Bacc(target_bir_lowering=False)

    xTb_d = nc.declare_dram_parameter("xTb", [D, T], bf, isOutput=False)
    x8_d = nc.declare_dram_parameter("x8p", [NDP * P, 2 * T], f8, isOutput=False)
    wk_d = nc.declare_dram_parameter("wk8p", [NDP * P, 2 * JG], f8, isOutput=False)
    wv_d = nc.declare_dram_parameter("wv8p", [NDP * P, 2 * JG], f8, isOutput=False)
    wq_d = nc.declare_dram_parameter("wq", [D, JG], bf, isOutput=False)
    wo_d = nc.declare_dram_parameter("wo", [JG, D], bf, isOutput=False)
    d8_d = nc.declare_dram_parameter("d8p", [NSP * P, 2 * T], f8, isOutput=False)
    bq_d = nc.declare_dram_parameter("bqT", [P, NJT], f32, isOutput=False)
    bv_d = nc.declare_dram_parameter("bvT", [P, NJT], f32, isOutput=False)
    out_d = nc.declare_dram_parameter("out", [T, D], f32, isOutput=True)

    with tile.TileContext(nc) as tc:
        with (
            tc.tile_pool(name="const", bufs=1) as cp,
            tc.tile_pool(name="ew", bufs=24) as ewp,
            tc.tile_pool(name="ps", bufs=8, space="PSUM") as pp,
            tc.tile_pool(name="tmp", bufs=4) as tp,
        ):
            # ---- constant loads (first K-proj deps hoisted to the top) ----
            x8_0 = cp.tile([P, 2, T], f8, tag="x8_0", name="x8_0")
            nc.sync.dma_start(x8_0[:], x8_d[0:P, :].rearrange("p (e t) -> p e t", e=2))
            wk8_0 = cp.tile([P, 2, JG], f8, tag="wk8_0", name="wk8_0")
            nc.sync.dma_start(wk8_0[:], wk_d[0:P, :].rearrange("p (e t) -> p e t", e=2))
            bq_sb = cp.tile([P, NJT], f32, tag="bq", name="bq")
            bv_sb = cp.tile([P, NJT], f32, tag="bv", name="bv")
            nc.sync.dma_start(bq_sb[:], bq_d[:])
            nc.sync.dma_start(bv_sb[:], bv_d[:])
            negln = cp.tile([P, 1], f32, tag="negln", name="negln")
            nc.vector.memset(negln[:], -LN_EKS)

            # paired fp8 x / W tiles (interleave loads: x pair, wk pair, ...)
            x8, wk8, wv8 = [x8_0], [wk8_0], []
            for i in range(1, NDP):
                tx = cp.tile([P, 2, T], f8, tag=f"x8_{i}", name=f"x8_{i}")
                nc.sync.dma_start(tx[:], x8_d[i * P:(i + 1) * P, :].rearrange(
                    "p (e t) -> p e t", e=2))
                x8.append(tx)
                tk = cp.tile([P, 2, JG], f8, tag=f"wk8_{i}", name=f"wk8_{i}")
                nc.sync.dma_start(tk[:], wk_d[i * P:(i + 1) * P, :].rearrange(
                    "p (e t) -> p e t", e=2))
                wk8.append(tk)
            for i in range(NDP):
                tv = cp.tile([P, 2, JG], f8, tag=f"wv8_{i}", name=f"wv8_{i}")
                nc.sync.dma_start(tv[:], wv_d[i * P:(i + 1) * P, :].rearrange(
                    "p (e t) -> p e t", e=2))
                wv8.append(tv)

            xTb = []
            for d in range(NDT):
                t_ = cp.tile([P, T], bf, tag=f"xTb{d}", name=f"xTb{d}")
                nc.sync.dma_start(t_[:], xTb_d[d * P:(d + 1) * P, :])
                xTb.append(t_)
            wq = []
            for d in range(NDT):
                t_ = cp.tile([P, JG], bf, tag=f"wq{d}", name=f"wq{d}")
                nc.sync.dma_start(t_[:], wq_d[d * P:(d + 1) * P, :])
                wq.append(t_)
            wo = []
            for i in range(NJT):
                t_ = cp.tile([P, D], bf, tag=f"wo{i}", name=f"wo{i}")
                nc.sync.dma_start(t_[:], wo_d[i * P:(i + 1) * P, :])
                wo.append(t_)

            eK8 = [cp.tile([P, 2, JG], f8, tag=f"eK8_{i}", name=f"eK8_{i}")
                   for i in range(NSP)]
            eKV8 = [cp.tile([P, 2, JG], f8, tag=f"eKV8_{i}", name=f"eKV8_{i}")
                    for i in range(NSP)]
            QT = [cp.tile([P, T], f32, tag=f"QT{j}", name=f"QT{j}") for j in range(NJT)]
            aftT = [cp.tile([P, T], bf, tag=f"aftT{j}", name=f"aftT{j}")
                    for j in range(NJT)]
            cn4_sb = cp.tile([P, NJT], f32, tag="cn4", name="cn4")
            cd_sb = cp.tile([P, NJT], f32, tag="cd", name="cd")
            cn_acc = cp.tile([P, JG], f32, tag="cn_acc", name="cn_acc")
            cd_acc = cp.tile([P, JG], f32, tag="cd_acc", name="cd_acc")
            cn_red = cp.tile([P, JG], f32, tag="cn_red", name="cn_red")
            cd_red = cp.tile([P, JG], f32, tag="cd_red", name="cd_red")

            # ---- K/V projections: fp8 DoubleRow, [s, j] natural layout ----
            # PSUM holds 16*K_true (W was pre-scaled x16).  K leads V by 4
            # t-tiles so V's first matmuls don't stall on the wv8 DMAs.
            def emit_kproj(tt):
                i8, e = divmod(tt, 2)
                ps_k = pp.tile([P, JG], f32, tag="ps", name="psv")
                for i in range(NDP):
                    nc.tensor.matmul(
                        ps_k[:],
                        x8[i][:, :, tt * P:(tt + 1) * P],
                        wk8[i][:],
                        start=(i == 0), stop=(i == NDP - 1),
                        perf_mode=DR,
                    )
                # eK8 = exp(psum/16)/EKS = exp(psum/16 - ln EKS)
                nc.scalar.activation(eK8[i8][:, e, :], ps_k[:], Act.Exp,
                                     bias=negln[:], scale=1.0 / WSCALE)
                if tt == 0:
                    nc.vector.tensor_copy(cd_acc[:], eK8[i8][:, e, :])
                else:
                    nc.vector.tensor_tensor(cd_acc[:], cd_acc[:],
                                            eK8[i8][:, e, :], Alu.add)

            def emit_vproj(tt):
                i8, e = divmod(tt, 2)
                ps_v = pp.tile([P, JG], f32, tag="ps", name="psv")
                for i in range(NDP):
                    nc.tensor.matmul(
                        ps_v[:],
                        x8[i][:, :, tt * P:(tt + 1) * P],
                        wv8[i][:],
                        start=(i == 0), stop=(i == NDP - 1),
                        perf_mode=DR,
                    )
                # eKV8 = eK8 * EKS * (psum/16) / EKVS = (psum/64) * eK8
                nc.vector.scalar_tensor_tensor(
                    eKV8[i8][:, e, :], ps_v[:],
                    EKS / (WSCALE * EKVS), eK8[i8][:, e, :],
                    Alu.mult, Alu.mult,
                )
                if tt == 0:
                    nc.vector.tensor_copy(cn_acc[:], eKV8[i8][:, e, :])
                else:
                    nc.vector.tensor_tensor(cn_acc[:], cn_acc[:],
                                            eKV8[i8][:, e, :], Alu.add)

            KV_LEAD = 4
            for tt in range(KV_LEAD):
                emit_kproj(tt)
            for tt in range(NTT):
                if tt + KV_LEAD < NTT:
                    emit_kproj(tt + KV_LEAD)
                emit_vproj(tt)

            # ---- column sums on gpsimd (PE stays on matmuls) --------------
            # per-slice partials accumulated on DVE above; one cross-partition
            # reduce each here, then scatter row 0 into per-partition vectors.
            from concourse import bass_isa
            nc.gpsimd.partition_all_reduce(cn_red[:], cn_acc[:], P,
                                           bass_isa.ReduceOp.add)
            nc.gpsimd.partition_all_reduce(cd_red[:], cd_acc[:], P,
                                           bass_isa.ReduceOp.add)
            cn_row = tp.tile([1, JG], f32, tag="cnr", name="cnr", bufs=1)
            # nn uses scale 4/DSCALE with bias 4*cn; dd uses 1/DSCALE with cd
            nc.scalar.mul(cn_row[:], cn_red[0:1, :], EKVS / EKS)
            for m in range(NJT):
                nc.sync.dma_start(cn4_sb[:, m:m + 1],
                                  cn_row[0:1, m * P:(m + 1) * P])
                nc.sync.dma_start(cd_sb[:, m:m + 1],
                                  cd_red[0:1, m * P:(m + 1) * P])

            # ---- Q projection (bf16), transposed: QT[j, t] = sigmoid(.) ---
            for jt in range(NJT):
                for c in range(NTC):
                    ps_q = pp.tile([P, NC_CHUNK], f32, tag="ps", name="psc")
                    for d in range(NDT):
                        nc.tensor.matmul(
                            ps_q[:],
                            wq[d][:, jt * P:(jt + 1) * P],
                            xTb[d][:, c * NC_CHUNK:(c + 1) * NC_CHUNK],
                            start=(d == 0), stop=(d == NDT - 1),
                        )
                    nc.scalar.activation(
                        QT[jt][:, c * NC_CHUNK:(c + 1) * NC_CHUNK],
                        ps_q[:], Act.Sigmoid, bias=bq_sb[:, jt:jt + 1],
                    )

            # ---- AFT delta matmuls (fp8 DoubleRow) + epilogue + outproj ---
            def emit_outproj(c):
                for tt in range(4 * c, 4 * (c + 1)):
                    for ic in range(NIC):
                        ps_y = pp.tile([P, NC_CHUNK], f32, tag="ps", name="psc")
                        for jt in range(NJT):
                            nc.tensor.matmul(
                                ps_y[:],
                                aftT[jt][:, tt * P:(tt + 1) * P],
                                wo[jt][:, ic * NC_CHUNK:(ic + 1) * NC_CHUNK],
                                start=(jt == 0), stop=(jt == NJT - 1),
                            )
                        y_sb = tp.tile([P, NC_CHUNK], f32, tag="y", name="ysb", bufs=6)
                        if (tt * NIC + ic) % 2 == 0:
                            nc.vector.tensor_copy(y_sb[:], ps_y[:])
                        else:
                            nc.scalar.copy(y_sb[:], ps_y[:])
                        nc.sync.dma_start(
                            out_d[tt * P:(tt + 1) * P,
                                  ic * NC_CHUNK:(ic + 1) * NC_CHUNK],
                            y_sb[:],
                        )

            for c in range(NTC):
                dw_c = []
                for i in range(NSP):
                    dw_t = ewp.tile([P, 2, NC_CHUNK], f8, tag="ewt", name="ewt")
                    nc.sync.dma_start(
                        dw_t[:],
                        d8_d[i * P:(i + 1) * P, :].rearrange(
                            "p (e t) -> p e t", e=2)[:, :, c * NC_CHUNK:(c + 1) * NC_CHUNK],
                    )
                    dw_c.append(dw_t)
                for half in range(2):
                    ms = (2 * half, 2 * half + 1)
                    ps_n = {m: pp.tile([P, NC_CHUNK], f32, tag="ps", name="psc")
                            for m in ms}
                    ps_dn = {m: pp.tile([P, NC_CHUNK], f32, tag="ps", name="psc")
                             for m in ms}
                    for i in range(NSP):
                        for m in ms:
                            nc.tensor.matmul(
                                ps_n[m][:],
                                eKV8[i][:, :, m * P:(m + 1) * P],
                                dw_c[i][:],
                                start=(i == 0), stop=(i == NSP - 1),
                                perf_mode=DR,
                            )
                            nc.tensor.matmul(
                                ps_dn[m][:],
                                eK8[i][:, :, m * P:(m + 1) * P],
                                dw_c[i][:],
                                start=(i == 0), stop=(i == NSP - 1),
                                perf_mode=DR,
                            )
                    for m in ms:
                        # nn = 4*num/EKVS = dnum*(4/DSCALE) + 4*cn
                        # dd = den/EKS    = dden*(1/DSCALE) + cd
                        # nn/dd = num/den exactly (EKVS/EKS = 4)
                        nn = tp.tile([P, NC_CHUNK], f32, tag="nn", name="nn", bufs=3)
                        nc.scalar.activation(nn[:], ps_n[m][:], Act.Identity,
                                             bias=cn4_sb[:, m:m + 1],
                                             scale=(EKVS / EKS) / DSCALE)
                        dd = tp.tile([P, NC_CHUNK], f32, tag="dd", name="dd", bufs=3)
                        nc.scalar.activation(dd[:], ps_dn[m][:], Act.Identity,
                                             bias=cd_sb[:, m:m + 1],
                                             scale=1.0 / DSCALE)
                        rcp = tp.tile([P, NC_CHUNK], f32, tag="rcp", name="rcp", bufs=3)
                        nc.vector.reciprocal_approx_fast(rcp[:], dd[:])
                        prod = tp.tile([P, NC_CHUNK], f32, tag="prod", name="prod", bufs=3)
                        nc.vector.tensor_tensor(prod[:], nn[:], rcp[:], Alu.mult)
                        # aftT = (num/den + bv) * sigmoid(QT)
                        nc.vector.scalar_tensor_tensor(
                            aftT[m][:, c * NC_CHUNK:(c + 1) * NC_CHUNK],
                            prod[:], bv_sb[:, m:m + 1],
                            QT[m][:, c * NC_CHUNK:(c + 1) * NC_CHUNK],
                            Alu.add, Alu.mult,
                        )
                if c > 0:
                    emit_outproj(c - 1)
            emit_outproj(NTC - 1)

    nc.compile()
    return nc


def _get_nc():
    global _NC
    if _NC is None:
        _NC = _build()
    return _NC


def _pair(a, npair):
    """[npair*256, W] -> [npair*128, 2*W] DoubleRow-paired layout."""
    w = a.shape[1]
    return np.ascontiguousarray(
        a.reshape(npair, 2, P, w).transpose(0, 2, 1, 3).reshape(npair * P, 2 * w))


def make_in_maps(inputs):
    bf16 = ml_dtypes.bfloat16
    f8 = ml_dtypes.float8_e4m3
    x = np.asarray(inputs["x"], dtype=np.float32)
    Wq = np.asarray(inputs["Wq"], dtype=np.float32)
    Wk = np.asarray(inputs["Wk"], dtype=np.float32)
    Wv = np.asarray(inputs["Wv"], dtype=np.float32)
    Wo = np.asarray(inputs["Wo"], dtype=np.float32)
    bq = np.asarray(inputs["bq"], dtype=np.float32)
    bv = np.asarray(inputs["bv"], dtype=np.float32)
    wbias = np.asarray(inputs["wbias"], dtype=np.float32)
    # bk cancels in num/den; bq, bv, bo handled explicitly.

    d8p = _pair(((np.exp(wbias) - 1.0) * DSCALE).T.astype(f8).astype(np.float32),
                NSP).astype(f8)

    in_maps = []
    for c in range(NCORES):
        b, g = divmod(c, G)
        sl = slice(g * JG, (g + 1) * JG)
        xT = np.ascontiguousarray(x[b].T)
        in_maps.append({
            "xTb": xT.astype(bf16),
            "x8p": _pair(xT.astype(f8).astype(np.float32), NDP).astype(f8),
            "wk8p": _pair(np.ascontiguousarray(Wk[:, sl]) * WSCALE, NDP).astype(f8),
            "wv8p": _pair(np.ascontiguousarray(Wv[:, sl]) * WSCALE, NDP).astype(f8),
            "wq": np.ascontiguousarray(Wq[:, sl]).astype(bf16),
            "wo": np.ascontiguousarray(Wo[sl, :]).astype(bf16),
            "d8p": d8p,
            "bqT": np.ascontiguousarray(bq[sl].reshape(NJT, P).T),
            "bvT": np.ascontiguousarray(bv[sl].reshape(NJT, P).T),
        })
    return in_maps


def kernel(**inputs):
    import os
    import sys
    # The bass kernel needs the axon/neuron jax backend (8 NeuronCores). If a
    # harness pinned jax to cpu for the reference and jax isn't imported yet,
    # unpin it for this process.
    if "jax" not in sys.modules and os.environ.get("JAX_PLATFORMS") == "cpu":
        del os.environ["JAX_PLATFORMS"]

    from concourse.bass_utils import run_bass_kernel_spmd

    bo = np.asarray(inputs["bo"], dtype=np.float32)
    in_maps = make_in_maps(inputs)
    res = run_bass_kernel_spmd(_get_nc(), in_maps, core_ids=list(range(NCORES)))
    parts = [res.results[c]["out"] for c in range(NCORES)]
    out = np.empty((B, T, D), dtype=np.float32)
    for b in range(B):
        out[b] = parts[G * b] + parts[G * b + 1] + bo[None, :]
    return out


# revision 30
# speedup vs baseline: 1.1803x; 1.0152x over previous
"""AFT-Full (Attention Free Transformer) on 8 Trainium2 NeuronCores.

Math (per batch b):
  Q = x@Wq+bq, K = x@Wk+bk, V = x@Wv+bv          (per-head reshape is a no-op
  num = ew @ (exp(K) * V), den = ew @ exp(K)      because ew is shared by all
  out = (sigmoid(Q) * num / den) @ Wo + bo        heads: ew = exp(wbias))

Identities used:
  - with biases bk, bv: num/den = num0/den0 + bv and bk cancels entirely.
  - ew = 1 + delta with |delta| <= 0.04 (wbias is xavier-small), so
    num = colsum(eKV) + delta @ eKV, den = colsum(eK) + delta @ eK.
    This lets delta (scaled x256) and eK/eKV live in fp8e4m3 while ew == 1
    to machine precision would have destroyed fp8's mantissa.

Sharding: 8 cores = 4 batches x 2 head-groups (512 features each).  Each
core computes a partial [T, D] output; the host adds the two group partials
per batch plus bo.

Precision plan (validated vs reference in numpy, rel err ~1.2e-2 global):
  - K/V projections + AFT delta-matmuls: fp8e4m3 with DoubleRow (2x rate).
    Scales: W x16 (away from denorms), eK /2, eKV /8, delta x256.
  - Q projection + out projection: bf16 (full rate).
  - All PSUM accumulation f32; epilogues f32; output f32.

DoubleRow pairs the contraction dim: lhsT/rhs are [128, 2, free] APs and
out = sum_e lhsT[:,e,:].T @ rhs[:,e,:].  The host pre-interleaves x, W and
delta into that paired layout; eK/eKV pair tiles are filled by the
projection drains (t-tile tt -> pair tt//2, slot tt%2).

Column sums are one ones-vector matmul pass -> [1, 512] PSUM, scattered to
per-partition [128, 4] vectors by small DMAs, then injected as ACT biases in
the AFT epilogue: ratio = (4/256 * dnum + 4*cn) / (1/256 * dden + cd) which
equals num/den exactly for the chosen scales (EKVS/EKS = 4).
"""

import numpy as np
import ml_dtypes

B, T, D, H = 4, 2048, 1024, 16
G = 2                  # head-groups (cores = B * G)
JG = D // G            # 512 features per group
NCORES = 8
P = 128                # partition tile
NDT = D // P           # 8  d-tiles
NDP = NDT // 2         # 4  paired d-tiles (DoubleRow)
NTT = T // P           # 16 t-tiles / s-tiles
NSP = NTT // 2         # 8  paired s-tiles
NJT = JG // P          # 4  j-tiles per group
NC_CHUNK = 512         # matmul moving free-dim (one PSUM bank of f32)
NTC = T // NC_CHUNK    # 4  t-chunks
NIC = D // NC_CHUNK    # 2  i-chunks of the final output

WSCALE = 16.0          # W pre-scale (host)
EKS = 2.0              # eK stored as eK/EKS
EKVS = 8.0             # eKV stored as eKV/EKVS
DSCALE = 256.0         # delta stored as delta*DSCALE
LN_EKS = float(np.log(EKS))

_NC = None             # cached compiled Bass graph


def _build():
    from concourse import bacc, mybir, tile

    dt = mybir.dt
    bf = dt.bfloat16
    f8 = dt.float8e4
    f32 = dt.float32
    Alu = mybir.AluOpType
    Act = mybir.ActivationFunctionType
    DR = mybir.MatmulPerfMode.DoubleRow

    nc = bacc.Bacc(target_bir_lowering=False)

    xTb_d = nc.declare_dram_parameter("xTb", [D, T], bf, isOutput=False)
    x8_d = nc.declare_dram_parameter("x8p", [NDP * P, 2 * T], f8, isOutput=False)
    wk_d = nc.declare_dram_parameter("wk8p", [NDP * P, 2 * JG], f8, isOutput=False)
    wv_d = nc.declare_dram_parameter("wv8p", [NDP * P, 2 * JG], f8, isOutput=False)
    wq_d = nc.declare_dram_parameter("wq", [D, JG], bf, isOutput=False)
    wo_d = nc.declare_dram_parameter("wo", [JG, D], bf, isOutput=False)
    d8_d = nc.declare_dram_parameter("d8p", [NSP * P, 2 * T], f8, isOutput=False)
    bq_d = nc.declare_dram_parameter("bqT", [P, NJT], f32, isOutput=False)
    bv_d = nc.declare_dram_parameter("bvT", [P, NJT], f32, isOutput=False)
    out_d = nc.declare_dram_parameter("out", [T, D], f32, isOutput=True)

    with tile.TileContext(nc) as tc:
        with (
            tc.tile_pool(name="const", bufs=1) as cp,
            tc.tile_pool(name="ew", bufs=28) as ewp,
            tc.tile_pool(name="ps", bufs=8, space="PSUM") as pp,
            tc.tile_pool(name="tmp", bufs=4) as tp,
        ):
            # ---- constant loads (first K-proj deps hoisted to the top) ----
            x8_0 = cp.tile([P, 2, T], f8, tag="x8_0", name="x8_0")
            nc.sync.dma_start(x8_0[:], x8_d[0:P, :].rearrange("p (e t) -> p e t", e=2))
            wk8_0 = cp.tile([P, 2, JG], f8, tag="wk8_0", name="wk8_0")
            nc.sync.dma_start(wk8_0[:], wk_d[0:P, :].rearrange("p (e t) -> p e t", e=2))
            bq_sb = cp.tile([P, NJT], f32, tag="bq", name="bq")
            bv_sb = cp.tile([P, NJT], f32, tag="bv", name="bv")
            nc.sync.dma_start(bq_sb[:], bq_d[:])
            nc.sync.dma_start(bv_sb[:], bv_d[:])
            negln = cp.tile([P, 1], f32, tag="negln", name="negln")
            nc.vector.memset(negln[:], -LN_EKS)

            # paired fp8 x / W tiles (interleave loads: x pair, wk pair, ...)
            x8, wk8, wv8 = [x8_0], [wk8_0], []
            for i in range(1, NDP):
                tx = cp.tile([P, 2, T], f8, tag=f"x8_{i}", name=f"x8_{i}")
                nc.sync.dma_start(tx[:], x8_d[i * P:(i + 1) * P, :].rearrange(
                    "p (e t) -> p e t", e=2))
                x8.append(tx)
                tk = cp.tile([P, 2, JG], f8, tag=f"wk8_{i}", name=f"wk8_{i}")
                nc.sync.dma_start(tk[:], wk_d[i * P:(i + 1) * P, :].rearrange(
                    "p (e t) -> p e t", e=2))
                wk8.append(tk)
            for i in range(NDP):
                tv = cp.tile([P, 2, JG], f8, tag=f"wv8_{i}", name=f"wv8_{i}")
                nc.sync.dma_start(tv[:], wv_d[i * P:(i + 1) * P, :].rearrange(
                    "p (e t) -> p e t", e=2))
                wv8.append(tv)

            xTb = []
            for d in range(NDT):
                t_ = cp.tile([P, T], bf, tag=f"xTb{d}", name=f"xTb{d}")
                nc.sync.dma_start(t_[:], xTb_d[d * P:(d + 1) * P, :])
                xTb.append(t_)
            wq = []
            for d in range(NDT):
                t_ = cp.tile([P, JG], bf, tag=f"wq{d}", name=f"wq{d}")
                nc.sync.dma_start(t_[:], wq_d[d * P:(d + 1) * P, :])
                wq.append(t_)
            wo = []
            for i in range(NJT):
                t_ = cp.tile([P, D], bf, tag=f"wo{i}", name=f"wo{i}")
                nc.sync.dma_start(t_[:], wo_d[i * P:(i + 1) * P, :])
                wo.append(t_)

            eK8 = [cp.tile([P, 2, JG], f8, tag=f"eK8_{i}", name=f"eK8_{i}")
                   for i in range(NSP)]
            eKV8 = [cp.tile([P, 2, JG], f8, tag=f"eKV8_{i}", name=f"eKV8_{i}")
                    for i in range(NSP)]
            QT = [cp.tile([P, T], bf, tag=f"QT{j}", name=f"QT{j}") for j in range(NJT)]
            aftT = [cp.tile([P, T], bf, tag=f"aftT{j}", name=f"aftT{j}")
                    for j in range(NJT)]
            cn4_sb = cp.tile([P, NJT], f32, tag="cn4", name="cn4")
            cd_sb = cp.tile([P, NJT], f32, tag="cd", name="cd")
            cn_acc = cp.tile([P, JG], f32, tag="cn_acc", name="cn_acc")
            cd_acc = cp.tile([P, JG], f32, tag="cd_acc", name="cd_acc")
            cn_red = cp.tile([P, JG], f32, tag="cn_red", name="cn_red")
            cd_red = cp.tile([P, JG], f32, tag="cd_red", name="cd_red")

            # ---- K/V projections: fp8 DoubleRow, [s, j] natural layout ----
            # PSUM holds 16*K_true (W was pre-scaled x16).  K leads V by 4
            # t-tiles so V's first matmuls don't stall on the wv8 DMAs.
            def emit_kproj(tt):
                i8, e = divmod(tt, 2)
                ps_k = pp.tile([P, JG], f32, tag="ps", name="psv")
                for i in range(NDP):
                    nc.tensor.matmul(
                        ps_k[:],
                        x8[i][:, :, tt * P:(tt + 1) * P],
                        wk8[i][:],
                        start=(i == 0), stop=(i == NDP - 1),
                        perf_mode=DR,
                    )
                # eK8 = exp(psum/16)/EKS = exp(psum/16 - ln EKS)
                nc.scalar.activation(eK8[i8][:, e, :], ps_k[:], Act.Exp,
                                     bias=negln[:], scale=1.0 / WSCALE)
                if tt == 0:
                    nc.vector.tensor_copy(cd_acc[:], eK8[i8][:, e, :])
                else:
                    nc.vector.tensor_tensor(cd_acc[:], cd_acc[:],
                                            eK8[i8][:, e, :], Alu.add)

            def emit_vproj(tt):
                i8, e = divmod(tt, 2)
                ps_v = pp.tile([P, JG], f32, tag="ps", name="psv")
                for i in range(NDP):
                    nc.tensor.matmul(
                        ps_v[:],
                        x8[i][:, :, tt * P:(tt + 1) * P],
                        wv8[i][:],
                        start=(i == 0), stop=(i == NDP - 1),
                        perf_mode=DR,
                    )
                # eKV8 = eK8 * EKS * (psum/16) / EKVS = (psum/64) * eK8
                nc.vector.scalar_tensor_tensor(
                    eKV8[i8][:, e, :], ps_v[:],
                    EKS / (WSCALE * EKVS), eK8[i8][:, e, :],
                    Alu.mult, Alu.mult,
                )
                if tt == 0:
                    nc.vector.tensor_copy(cn_acc[:], eKV8[i8][:, e, :])
                else:
                    nc.vector.tensor_tensor(cn_acc[:], cn_acc[:],
                                            eKV8[i8][:, e, :], Alu.add)

            KV_LEAD = 4
            for tt in range(KV_LEAD):
                emit_kproj(tt)
            for tt in range(NTT):
                if tt + KV_LEAD < NTT:
                    emit_kproj(tt + KV_LEAD)
                emit_vproj(tt)

            # ---- column sums on gpsimd (PE stays on matmuls) --------------
            # per-slice partials accumulated on DVE above; one cross-partition
            # reduce each here, then scatter row 0 into per-partition vectors.
            from concourse import bass_isa
            nc.gpsimd.partition_all_reduce(cn_red[:], cn_acc[:], P,
                                           bass_isa.ReduceOp.add)
            nc.gpsimd.partition_all_reduce(cd_red[:], cd_acc[:], P,
                                           bass_isa.ReduceOp.add)
            cn_row = tp.tile([1, JG], f32, tag="cnr", name="cnr", bufs=1)
            # nn uses scale 4/DSCALE with bias 4*cn; dd uses 1/DSCALE with cd
            nc.scalar.mul(cn_row[:], cn_red[0:1, :], EKVS / EKS)
            for m in range(NJT):
                nc.sync.dma_start(cn4_sb[:, m:m + 1],
                                  cn_row[0:1, m * P:(m + 1) * P])
                nc.sync.dma_start(cd_sb[:, m:m + 1],
                                  cd_red[0:1, m * P:(m + 1) * P])

            # ---- Q projection (bf16), transposed: QT[j, t] = sigmoid(.) ---
            for jt in range(NJT):
                for c in range(NTC):
                    ps_q = pp.tile([P, NC_CHUNK], f32, tag="ps", name="psc")
                    for d in range(NDT):
                        nc.tensor.matmul(
                            ps_q[:],
                            wq[d][:, jt * P:(jt + 1) * P],
                            xTb[d][:, c * NC_CHUNK:(c + 1) * NC_CHUNK],
                            start=(d == 0), stop=(d == NDT - 1),
                        )
                    nc.scalar.activation(
                        QT[jt][:, c * NC_CHUNK:(c + 1) * NC_CHUNK],
                        ps_q[:], Act.Sigmoid, bias=bq_sb[:, jt:jt + 1],
                    )

            # ---- AFT delta matmuls (fp8 DoubleRow) + epilogue + outproj ---
            def emit_outproj(c):
                for tt in range(4 * c, 4 * (c + 1)):
                    for ic in range(NIC):
                        ps_y = pp.tile([P, NC_CHUNK], f32, tag="ps", name="psc")
                        for jt in range(NJT):
                            nc.tensor.matmul(
                                ps_y[:],
                                aftT[jt][:, tt * P:(tt + 1) * P],
                                wo[jt][:, ic * NC_CHUNK:(ic + 1) * NC_CHUNK],
                                start=(jt == 0), stop=(jt == NJT - 1),
                            )
                        y_sb = tp.tile([P, NC_CHUNK], f32, tag="y", name="ysb", bufs=8)
                        if (tt * NIC + ic) % 2 == 0:
                            nc.vector.tensor_copy(y_sb[:], ps_y[:])
                        else:
                            nc.scalar.copy(y_sb[:], ps_y[:])
                        nc.sync.dma_start(
                            out_d[tt * P:(tt + 1) * P,
                                  ic * NC_CHUNK:(ic + 1) * NC_CHUNK],
                            y_sb[:],
                        )

            for c in range(NTC):
                dw_c = []
                for i in range(NSP):
                    dw_t = ewp.tile([P, 2, NC_CHUNK], f8, tag="ewt", name="ewt")
                    nc.sync.dma_start(
                        dw_t[:],
                        d8_d[i * P:(i + 1) * P, :].rearrange(
                            "p (e t) -> p e t", e=2)[:, :, c * NC_CHUNK:(c + 1) * NC_CHUNK],
                    )
                    dw_c.append(dw_t)
                for half in range(2):
                    ms = (2 * half, 2 * half + 1)
                    ps_n = {m: pp.tile([P, NC_CHUNK], f32, tag="ps", name="psc")
                            for m in ms}
                    ps_dn = {m: pp.tile([P, NC_CHUNK], f32, tag="ps", name="psc")
                             for m in ms}
                    for i in range(NSP):
                        for m in ms:
                            nc.tensor.matmul(
                                ps_n[m][:],
                                eKV8[i][:, :, m * P:(m + 1) * P],
                                dw_c[i][:],
                                start=(i == 0), stop=(i == NSP - 1),
                                perf_mode=DR,
                            )
                            nc.tensor.matmul(
                                ps_dn[m][:],
                                eK8[i][:, :, m * P:(m + 1) * P],
                                dw_c[i][:],
                                start=(i == 0), stop=(i == NSP - 1),
                                perf_mode=DR,
                            )
                    for m in ms:
                        # nn = 4*num/EKVS = dnum*(4/DSCALE) + 4*cn
                        # dd = den/EKS    = dden*(1/DSCALE) + cd
                        # nn/dd = num/den exactly (EKVS/EKS = 4)
                        nn = tp.tile([P, NC_CHUNK], f32, tag="nn", name="nn", bufs=4)
                        nc.scalar.activation(nn[:], ps_n[m][:], Act.Identity,
                                             bias=cn4_sb[:, m:m + 1],
                                             scale=(EKVS / EKS) / DSCALE)
                        dd = tp.tile([P, NC_CHUNK], f32, tag="dd", name="dd", bufs=4)
                        nc.scalar.activation(dd[:], ps_dn[m][:], Act.Identity,
                                             bias=cd_sb[:, m:m + 1],
                                             scale=1.0 / DSCALE)
                        rcp = tp.tile([P, NC_CHUNK], f32, tag="rcp", name="rcp", bufs=4)
                        nc.vector.reciprocal_approx_fast(rcp[:], dd[:])
                        prod = tp.tile([P, NC_CHUNK], f32, tag="prod", name="prod", bufs=4)
                        nc.vector.tensor_tensor(prod[:], nn[:], rcp[:], Alu.mult)
                        # aftT = (num/den + bv) * sigmoid(QT)
                        nc.vector.scalar_tensor_tensor(
                            aftT[m][:, c * NC_CHUNK:(c + 1) * NC_CHUNK],
                            prod[:], bv_sb[:, m:m + 1],
                            QT[m][:, c * NC_CHUNK:(c + 1) * NC_CHUNK],
                            Alu.add, Alu.mult,
                        )
                if c > 0:
                    emit_outproj(c - 1)
            emit_outproj(NTC - 1)

    nc.compile()
    return nc


def _get_nc():
    global _NC
    if _NC is None:
        _NC = _build()
    return _NC


def _pair(a, npair):
    """[npair*256, W] -> [npair*128, 2*W] DoubleRow-paired layout."""
    w = a.shape[1]
    return np.ascontiguousarray(
        a.reshape(npair, 2, P, w).transpose(0, 2, 1, 3).reshape(npair * P, 2 * w))


def make_in_maps(inputs):
    bf16 = ml_dtypes.bfloat16
    f8 = ml_dtypes.float8_e4m3
    x = np.asarray(inputs["x"], dtype=np.float32)
    Wq = np.asarray(inputs["Wq"], dtype=np.float32)
    Wk = np.asarray(inputs["Wk"], dtype=np.float32)
    Wv = np.asarray(inputs["Wv"], dtype=np.float32)
    Wo = np.asarray(inputs["Wo"], dtype=np.float32)
    bq = np.asarray(inputs["bq"], dtype=np.float32)
    bv = np.asarray(inputs["bv"], dtype=np.float32)
    wbias = np.asarray(inputs["wbias"], dtype=np.float32)
    # bk cancels in num/den; bq, bv, bo handled explicitly.

    d8p = _pair(((np.exp(wbias) - 1.0) * DSCALE).T.astype(f8).astype(np.float32),
                NSP).astype(f8)

    in_maps = []
    for c in range(NCORES):
        b, g = divmod(c, G)
        sl = slice(g * JG, (g + 1) * JG)
        xT = np.ascontiguousarray(x[b].T)
        in_maps.append({
            "xTb": xT.astype(bf16),
            "x8p": _pair(xT.astype(f8).astype(np.float32), NDP).astype(f8),
            "wk8p": _pair(np.ascontiguousarray(Wk[:, sl]) * WSCALE, NDP).astype(f8),
            "wv8p": _pair(np.ascontiguousarray(Wv[:, sl]) * WSCALE, NDP).astype(f8),
            "wq": np.ascontiguousarray(Wq[:, sl]).astype(bf16),
            "wo": np.ascontiguousarray(Wo[sl, :]).astype(bf16),
            "d8p": d8p,
            "bqT": np.ascontiguousarray(bq[sl].reshape(NJT, P).T),
            "bvT": np.ascontiguousarray(bv[sl].reshape(NJT, P).T),
        })
    return in_maps


def kernel(**inputs):
    import os
    import sys
    # The bass kernel needs the axon/neuron jax backend (8 NeuronCores). If a
    # harness pinned jax to cpu for the reference and jax isn't imported yet,
    # unpin it for this process.
    if "jax" not in sys.modules and os.environ.get("JAX_PLATFORMS") == "cpu":
        del os.environ["JAX_PLATFORMS"]

    from concourse.bass_utils import run_bass_kernel_spmd

    bo = np.asarray(inputs["bo"], dtype=np.float32)
    in_maps = make_in_maps(inputs)
    res = run_bass_kernel_spmd(_get_nc(), in_maps, core_ids=list(range(NCORES)))
    parts = [res.results[c]["out"] for c in range(NCORES)]
    out = np.empty((B, T, D), dtype=np.float32)
    for b in range(B):
        out[b] = parts[G * b] + parts[G * b + 1] + bo[None, :]
    return out
